# revision 1
# baseline (speedup 1.0000x reference)
"""MS-Deformable-Attention Trainium2 kernel.

Sharding: 8 cores = 2 batches x 4 query-slices (5440 q each, padded to 5504).
No cross-core communication; host concatenates per-core output slices.

Per-core pipeline:
  A) value = input_flatten @ W_val + b_val (PE), then build v4 tables: for
     each (head m, level l, cell i) a 256B fp16 row holding all 4 bilinear
     corner values [k=4, ch=32] (corner shifts via identity-sliced PE
     matmuls). Levels 0/1 are per-head tables, levels 2/3 pack all heads
     (so every dma_gather int16 index stays < 32768).
  B) offsets/attn projection (PE, ref-points + bias folded in as extra lhsT
     rows), softmax (ACT exp + DVE reduce), pixel coords -> clamped corner
     base, fractional corner weights, gather indices. Indices are folded
     into dma_gather's 16-partition-wrapped int16 layout with two PE
     transposes per q-tile, then replicated across the 8 Q7 groups.
  C) one 256B dma_gather row per sample (q,m,l,p), then weighted corner
     reduce on DVE (broadcast mult + pair-add tree) into acc.
  D) PE transpose of acc; out = accT.T @ W_out + b_out; DMA out.

Gather order per (m,l,chunk): i = 128*(4*tl + p) + r  (r = q%128), so
gathered row i lands at out[r, 4*tl+p] and the idx tensor holds idx(r,tl,p)
at [r%16, 8*(4*tl+p) + r//16] -- exactly the (i%16, i//16) wrap.
"""
import os
import threading

import numpy as np

import concourse.bass as bass
import concourse.mybir as mybir
import concourse.tile as tile
import concourse.bacc as bacc
from concourse.masks import make_identity
from contextlib import ExitStack

SPATIAL = ((128, 128), (64, 64), (32, 32), (16, 16))
STARTS = (0, 16384, 20480, 21504)
NBLK = (128, 32, 8, 2)            # 128-row blocks per level
M, L, NP, D, C = 8, 4, 4, 32, 256
LQ, N = 21760, 2
QS, QP, NT = 5440, 5504, 43       # q-slice, padded, q-tiles
CH = 4                            # q-tiles per phase-C chunk
NCHUNK = (NT + CH - 1) // CH      # 11
GBLK = 170
P = 128
f32, f16, i16 = mybir.dt.float32, mybir.dt.float16, mybir.dt.int16
A = mybir.AluOpType
AF = mybir.ActivationFunctionType


def _b(ap, n):
    """Append a broadcast (step-0) innermost dim of size n to an AP."""
    return bass.AP(ap.tensor, ap.offset, list(ap.ap) + [[0, n]])


def _v(ap, off, dims):
    """SBUF view: keep partition dim, replace free dims ([step, count] in
    elements), add `off` elements."""
    return bass.AP(ap.tensor, ap.offset + off,
                   [list(ap.ap[0])] + [list(d) for d in dims])


def _dv(ap, off, dims):
    """DRAM view: replace ALL dims."""
    return bass.AP(ap.tensor, ap.offset + off, [list(d) for d in dims])


def _pv(ap, p0, pcnt, off, dims):
    """SBUF view with partition sub-range [p0, p0+pcnt) and explicit free
    dims."""
    pstep = ap.ap[0][0]
    return bass.AP(ap.tensor, ap.offset + p0 * pstep + off,
                   [[pstep, pcnt]] + [list(d) for d in dims])


def _glvl(g):
    for l in range(L):
        if g < STARTS[l] // P + NBLK[l]:
            return l, g - STARTS[l] // P
    raise ValueError(g)


def build_nc():
    nc = bacc.Bacc("TRN2")
    with tile.TileContext(nc) as tc:
        with tc.tile_pool(name="dram", bufs=1, space="DRAM") as dram:
            qa = dram.tile([264, QP], f32, kind="ExternalInput", uniquify=False, name="qa")
            ia = dram.tile([C, LQ], f32, kind="ExternalInput", uniquify=False, name="ia")
            wc = dram.tile([264, 384], f32, kind="ExternalInput", uniquify=False, name="wc")
            wcb = dram.tile([1, 384], f32, kind="ExternalInput", uniquify=False, name="wcb")
            wv = dram.tile([C, C], f32, kind="ExternalInput", uniquify=False, name="wv")
            bv = dram.tile([1, C], f32, kind="ExternalInput", uniquify=False, name="bv")
            wo = dram.tile([C, C], f32, kind="ExternalInput", uniquify=False, name="wo")
            bo = dram.tile([1, C], f32, kind="ExternalInput", uniquify=False, name="bo")
            out = dram.tile([QP, C], f32, kind="ExternalOutput", uniquify=False, name="out")
            # gather tables: l0/l1 per-head, l2/l3 all-head (int16 idx limit)
            v4t01 = [[dram.tile([SPATIAL[l][0] * SPATIAL[l][1], 128], f16,
                                uniquify=False, name=f"v4_{l}_{m}")
                      for m in range(M)] for l in (0, 1)]
            v4t23 = [dram.tile([M * SPATIAL[l][0] * SPATIAL[l][1], 128], f16,
                               uniquify=False, name=f"v4_{l}")
                     for l in (2, 3)]

            def v4_tensor(l, m):
                return v4t01[l][m] if l < 2 else v4t23[l - 2]

            with ExitStack() as ctx:
                const = ctx.enter_context(tc.tile_pool(name="const", bufs=1))
                io = ctx.enter_context(tc.tile_pool(name="io", bufs=3))
                vwin = ctx.enter_context(tc.tile_pool(name="vwin", bufs=4))
                v4p = ctx.enter_context(tc.tile_pool(name="v4p", bufs=2))
                pb = ctx.enter_context(tc.tile_pool(name="pb", bufs=2))
                w4p = ctx.enter_context(tc.tile_pool(name="w4p", bufs=2))
                gb = ctx.enter_context(tc.tile_pool(name="gb", bufs=3))
                mac = ctx.enter_context(tc.tile_pool(name="mac", bufs=2))
                accp = ctx.enter_context(tc.tile_pool(name="accp", bufs=1))
                psv = ctx.enter_context(tc.tile_pool(name="psv", bufs=1, space="PSUM"))
                psv4 = ctx.enter_context(tc.tile_pool(name="psv4", bufs=1, space="PSUM"))
                psb = ctx.enter_context(tc.tile_pool(name="psb", bufs=2, space="PSUM"))
                pstA = ctx.enter_context(tc.tile_pool(name="pstA", bufs=1, space="PSUM"))
                pstB = ctx.enter_context(tc.tile_pool(name="pstB", bufs=2, space="PSUM"))
                pso = ctx.enter_context(tc.tile_pool(name="pso", bufs=1, space="PSUM"))

                # ---------- constants ----------
                id16 = const.tile([P, P], f16)
                make_identity(nc, id16)
                id32 = const.tile([P, P], f32)
                make_identity(nc, id32)
                ones1 = const.tile([1, P], f32)
                nc.any.memset(ones1[:], 1.0)
                cpos1 = const.tile([P, 1], f32)
                nc.any.memset(cpos1[:], 1.0)
                cneg1 = const.tile([P, 1], f32)
                nc.any.memset(cneg1[:], -1.0)

                # shifted identities for the corner-shift matmuls:
                # SH[poff][k,p]=1 iff k=p+poff ; SH2[poff][k,p]=1 iff
                # k=p-(128-poff) (the next-block wrap part)
                shm, shm2 = {}, {}
                for poff in (1, 16, 17, 32, 33, 64, 65):
                    s1 = const.tile([P, P], f16, name=f"sh_{poff}")
                    nc.gpsimd.memset(s1[:], 0.0)
                    nc.gpsimd.affine_select(
                        out=s1[:], in_=s1[:],
                        compare_op=A.not_equal, fill=1.0, base=-poff,
                        pattern=[[-1, P]], channel_multiplier=1)
                    shm[poff] = s1
                    s2 = const.tile([P, P], f16, name=f"sh2_{poff}")
                    nc.gpsimd.memset(s2[:], 0.0)
                    nc.gpsimd.affine_select(
                        out=s2[:], in_=s2[:],
                        compare_op=A.not_equal, fill=1.0, base=P - poff,
                        pattern=[[-1, P]], channel_multiplier=1)
                    shm2[poff] = s2

                lim = const.tile([P, 256], f32)
                for l, (h, w) in enumerate(SPATIAL):
                    nc.any.memset(lim[:, l * 64: l * 64 + 32], float(w - 2))
                    nc.any.memset(lim[:, l * 64 + 32: l * 64 + 64], float(h - 2))
                # head offset into combined l2/l3 tables per idx-col (l,m,p)
                moff = const.tile([P, 128], f32)
                nc.any.memset(moff[:, 0:64], 0.0)
                for l in (2, 3):
                    h, w = SPATIAL[l]
                    for m in range(M):
                        nc.any.memset(
                            moff[:, l * 32 + m * 4: l * 32 + m * 4 + 4],
                            float(m * h * w))

                wv_sb = const.tile([P, 2, C], f32)
                nc.sync.dma_start(wv_sb[:, 0, :], wv[0:128, :])
                nc.sync.dma_start(wv_sb[:, 1, :], wv[128:256, :])
                bv_sb = const.tile([1, C], f32)
                nc.sync.dma_start(bv_sb[:], bv[:])
                wc_sb = const.tile([P, 2, 384], f32)
                nc.sync.dma_start(wc_sb[:, 0, :], wc[0:128, :])
                nc.sync.dma_start(wc_sb[:, 1, :], wc[128:256, :])
                wcr_sb = const.tile([8, 384], f32)
                nc.sync.dma_start(wcr_sb[:], wc[256:264, :])
                wcb_sb = const.tile([1, 384], f32)
                nc.sync.dma_start(wcb_sb[:], wcb[:])
                wo_sb = const.tile([P, 2, C], f32)
                nc.sync.dma_start(wo_sb[:, 0, :], wo[0:128, :])
                nc.sync.dma_start(wo_sb[:, 1, :], wo[128:256, :])
                bo_sb = const.tile([1, C], f32)
                nc.sync.dma_start(bo_sb[:], bo[:])

                # ---------- phase A: value proj + v4 build ----------
                vts = []

                def compute_value(b):
                    it = io.tile([P, 2, P], f32, tag="ia_t")
                    nc.sync.dma_start(it[:, 0, :], ia[0:128, b * P:(b + 1) * P])
                    nc.sync.dma_start(it[:, 1, :], ia[128:256, b * P:(b + 1) * P])
                    ps = psv.tile([P, C], f32, tag="psV")
                    nc.tensor.matmul(ps[:], it[:, 0, :], wv_sb[:, 0, :],
                                     start=True, stop=False)
                    nc.tensor.matmul(ps[:], it[:, 1, :], wv_sb[:, 1, :],
                                     start=False, stop=False)
                    nc.tensor.matmul(ps[:], ones1[:], bv_sb[:],
                                     start=False, stop=True)
                    vt = vwin.tile([P, C], f16, tag="vt")
                    nc.scalar.activation(vt[:], ps[:], AF.Copy)
                    vts.append(vt)

                v4g_holder = [None]

                def build_v4(g):
                    """Fill v4 group tile for global block g; emit group DMA
                    when the group completes."""
                    l, j = _glvl(g)
                    h, w = SPATIAL[l]
                    grp = 4 if l < 2 else 1
                    bl = j % grp
                    if bl == 0:
                        v4g_holder[0] = v4p.tile([P, grp, M, 4, D], f16,
                                                 tag="v4g", name="v4g")
                    v4g = v4g_holder[0]
                    for half in range(2):
                        ps4 = psv4.tile([P, 4, 4, D], f32, tag="ps4")
                        rhs_cols = slice(half * 4 * D, half * 4 * D + 4 * D)
                        for k, (ky, kx) in enumerate(
                                [(0, 0), (0, 1), (1, 0), (1, 1)]):
                            dlt = ky * w + kx
                            joff, poff = divmod(dlt, P)
                            src1 = vts[min(g + joff, GBLK - 1)]
                            if poff == 0:
                                nc.tensor.matmul(
                                    ps4[:, k, :, :], id16[:],
                                    src1[:, rhs_cols],
                                    start=True, stop=True)
                            else:
                                src2 = vts[min(g + joff + 1, GBLK - 1)]
                                nc.tensor.matmul(
                                    ps4[:, k, :, :], shm[poff][:],
                                    src1[:, rhs_cols],
                                    start=True, stop=False)
                                nc.tensor.matmul(
                                    ps4[:, k, :, :], shm2[poff][:],
                                    src2[:, rhs_cols],
                                    start=False, stop=True)
                        # psum (k, m4, ch) -> group tile (m4, k, ch)
                        nc.scalar.activation(
                            _v(v4g[:], bl * M * 4 * D + half * 4 * 4 * D,
                               [[4 * D, 4], [D, 4], [1, D]]),
                            _v(ps4[:], 0, [[D, 4], [4 * D, 4], [1, D]]),
                            AF.Copy)
                    if bl == grp - 1:
                        j0 = j - bl
                        if l < 2:
                            for m in range(M):
                                nc.sync.dma_start(
                                    _dv(v4_tensor(l, m)[:], j0 * P * P,
                                        [[P, P], [P * P, grp], [1, 4 * D]]),
                                    _v(v4g[:], m * 4 * D,
                                       [[M * 4 * D, grp], [1, 4 * D]]))
                        else:
                            nc.sync.dma_start(
                                _dv(v4_tensor(l, 0)[:], j0 * P * P,
                                    [[P, P], [h * w * P, M], [1, 4 * D]]),
                                _v(v4g[:], 0, [[4 * D, M], [1, 4 * D]]))

                for g in range(GBLK):
                    compute_value(g)
                    if g >= 2:
                        build_v4(g - 2)
                build_v4(GBLK - 2)
                build_v4(GBLK - 1)

                # ---------- phases B/C interleaved per chunk ----------
                acc = accp.tile([P, NT, M, D], f32)

                for c in range(NCHUNK):
                    t0, t1n = c * CH, min((c + 1) * CH, NT)
                    nt_c = t1n - t0
                    # idxw: 16-wrapped idx tensor, replicated to 128 parts.
                    # free layout (l, m, tl, p, u): per-(m,l) slice contiguous
                    idxm = w4p.tile([16, L * M * CH * NP * 8], i16, tag="idxm")
                    idxw = w4p.tile([P, L, M, CH, NP, 8], i16, tag="idxw")
                    if nt_c < CH:
                        nc.vector.memset(idxm[:], 0)
                    w4c = w4p.tile([P, CH, 128, 4], f16, tag="w4c")

                    for t in range(t0, t1n):
                        tl = t - t0
                        cs = slice(t * P, (t + 1) * P)
                        qt = io.tile([P, 2, P], f32, tag="qa_t")
                        nc.sync.dma_start(qt[:, 0, :], qa[0:128, cs])
                        nc.sync.dma_start(qt[:, 1, :], qa[128:256, cs])
                        rt = io.tile([8, P], f32, tag="ref_t")
                        nc.sync.dma_start(rt[:], qa[256:264, cs])

                        psB = psb.tile([P, 384], f32, tag="psB")
                        nc.tensor.matmul(psB[:], qt[:, 0, :], wc_sb[:, 0, :],
                                         start=True, stop=False)
                        nc.tensor.matmul(psB[:], qt[:, 1, :], wc_sb[:, 1, :],
                                         start=False, stop=False)
                        nc.tensor.matmul(psB[:], rt[:], wcr_sb[:],
                                         start=False, stop=False)
                        nc.tensor.matmul(psB[:], ones1[:], wcb_sb[:],
                                         start=False, stop=True)

                        # softmax over (l,p) per m; attn cols 256+(l,m,p)
                        ex = pb.tile([P, 128], f32, tag="ex")
                        nc.scalar.activation(ex[:], psB[:, 256:384], AF.Exp)
                        r1 = pb.tile([P, 32], f32, tag="r1")  # (l,m)
                        nc.vector.tensor_reduce(
                            out=r1[:],
                            in_=_v(ex[:], 0, [[32, 4], [4, 8], [1, 4]]),
                            op=A.add, axis=mybir.AxisListType.X)
                        r2 = pb.tile([P, 8], f32, tag="r2")   # (m)
                        nc.vector.tensor_reduce(
                            out=r2[:], in_=_v(r1[:], 0, [[1, 8], [8, 4]]),
                            op=A.add, axis=mybir.AxisListType.X)
                        rc = pb.tile([P, 8], f32, tag="rc")
                        nc.vector.reciprocal(rc[:], r2[:])
                        at = pb.tile([P, 128], f32, tag="at")  # (l,m,p)
                        nc.vector.tensor_tensor(
                            out=at[:], in0=ex[:],
                            in1=_v(rc[:], 0, [[0, 4], [1, 8], [0, 4]]),
                            op=A.mult)

                        # coords: cols (l,xy,m,p)
                        xcl = pb.tile([P, 256], f32, tag="xcl")
                        nc.vector.tensor_scalar(
                            out=xcl[:], in0=psB[:, 0:256], scalar1=0.0,
                            scalar2=None, op0=A.max)
                        xc = pb.tile([P, 256], f32, tag="xc")
                        nc.vector.tensor_tensor(out=xc[:], in0=xcl[:],
                                                in1=lim[:], op=A.min)
                        # floor(xc) via round-to-nearest (+-2^23) then
                        # fix-up: bs0 > xc  =>  bs0 - 1
                        bs0 = pb.tile([P, 256], f32, tag="bs0")
                        nc.vector.tensor_scalar(
                            out=bs0[:], in0=xc[:], scalar1=8388608.0,
                            scalar2=-8388608.0, op0=A.add, op1=A.add)
                        cmpg = pb.tile([P, 256], f32, tag="cmpg")
                        nc.vector.tensor_tensor(out=cmpg[:], in0=bs0[:],
                                                in1=xc[:], op=A.is_gt)
                        bs = pb.tile([P, 256], f32, tag="bs")
                        nc.vector.tensor_tensor(out=bs[:], in0=bs0[:],
                                                in1=cmpg[:], op=A.subtract)
                        f = pb.tile([P, 256], f32, tag="f")
                        nc.vector.tensor_tensor(out=f[:], in0=psB[:, 0:256],
                                                in1=bs[:], op=A.subtract)
                        ab0 = pb.tile([P, 256], f32, tag="ab0")
                        nc.scalar.activation(ab0[:], f[:], AF.Abs)
                        w0 = pb.tile([P, 256], f32, tag="w0")
                        nc.scalar.activation(w0[:], ab0[:], AF.Relu,
                                             scale=-1.0, bias=cpos1[:])
                        ab1 = pb.tile([P, 256], f32, tag="ab1")
                        nc.scalar.activation(ab1[:], f[:], AF.Abs,
                                             bias=cneg1[:])
                        w1 = pb.tile([P, 256], f32, tag="w1")
                        nc.scalar.activation(w1[:], ab1[:], AF.Relu,
                                             scale=-1.0, bias=cpos1[:])

                        byw = pb.tile([P, 128], f32, tag="byw")
                        for l, (h, w) in enumerate(SPATIAL):
                            nc.vector.tensor_scalar(
                                out=byw[:, l * 32:(l + 1) * 32],
                                in0=bs[:, l * 64 + 32: l * 64 + 64],
                                scalar1=float(w), scalar2=None, op0=A.mult)
                        idxf = pb.tile([P, 128], f32, tag="idxf")
                        nc.vector.tensor_tensor(
                            out=idxf[:],
                            in0=_v(bs[:], 0, [[64, 4], [1, 32]]),
                            in1=byw[:], op=A.add)
                        idxf2 = pb.tile([P, 128], f32, tag="idxf2")
                        nc.vector.tensor_tensor(out=idxf2[:], in0=idxf[:],
                                                in1=moff[:], op=A.add)

                        # fold idx into 16-wrap: transpose, then 8 16-col
                        # slice transposes put r%16 on partitions
                        psT = pstA.tile([P, P], f32, tag="psT")
                        nc.tensor.transpose(psT[:], idxf2[:], id32[:])
                        idxT = pb.tile([P, P], f32, tag="idxT")
                        nc.scalar.activation(idxT[:], psT[:], AF.Copy)
                        for u in range(8):
                            psU = pstB.tile([16, P], f32, tag="psU")
                            nc.tensor.transpose(
                                psU[:], idxT[:, 16 * u:16 * (u + 1)], id32[:])
                            # psU[r2,(l,m,p)] -> idxm[r2, (l, m, tl, p, u)]
                            nc.vector.tensor_copy(
                                out=_pv(idxm[:], 0, 16, tl * NP * 8 + u,
                                        [[M * CH * NP * 8, 4],
                                         [CH * NP * 8, 8], [8, 4]]),
                                in_=_v(psU[:], 0, [[32, 4], [4, 8], [1, 4]]))

                        # w4[(l,m,p), k=(ky,kx)] = wy_ky*wx_kx*attn (fp16)
                        for ky in range(2):
                            wyt = (w0, w1)[ky]
                            for kx in range(2):
                                wxt = (w0, w1)[kx]
                                tmp = pb.tile([P, 128], f32, tag="tmp")
                                nc.vector.tensor_tensor(
                                    out=tmp[:],
                                    in0=_v(wyt[:], 32, [[64, 4], [1, 32]]),
                                    in1=_v(wxt[:], 0, [[64, 4], [1, 32]]),
                                    op=A.mult)
                                nc.vector.tensor_tensor(
                                    out=_v(w4c[:], tl * 512 + ky * 2 + kx,
                                           [[4, 128]]),
                                    in0=tmp[:], in1=at[:], op=A.mult)

                    # replicate idx wrap to all 8 Q7 partition groups
                    for r in range(8):
                        nc.sync.dma_start(
                            _pv(idxw[:], 16 * r, 16, 0, [[1, 4096]]),
                            _pv(idxm[:], 0, 16, 0, [[1, 4096]]))

                    # ---- phase C: gathers + weighted reduce ----
                    ni = nt_c * 512
                    for l in range(L):
                        for m in range(M):
                            gbuf = gb.tile([P, CH * NP, 128], f16, tag="gbuf")
                            # Q7 idx scratch caps num_idxs at 1024/call
                            for hfi in range(0, nt_c, 2):
                                nth = min(2, nt_c - hfi)
                                nih = nth * 512
                                nc.gpsimd.dma_gather(
                                    gbuf[:, hfi * NP:(hfi + nth) * NP, :],
                                    v4_tensor(l, m)[:],
                                    _v(idxw[:],
                                       (l * M + m) * CH * NP * 8 + hfi * 32,
                                       [[1, nth * 32]]),
                                    nih, nih, 128)
                            prod = mac.tile([P, CH * NP, 4, D], f16, tag="prod")
                            nc.vector.tensor_tensor(
                                out=_v(prod[:], 0,
                                       [[512, nt_c], [128, 4], [32, 4], [1, D]]),
                                in0=_v(gbuf[:], 0,
                                       [[512, nt_c], [128, 4], [32, 4], [1, D]]),
                                in1=_b(_v(w4c[:], (l * 32 + m * 4) * 4,
                                          [[512, nt_c], [4, 4], [1, 4]]), D),
                                op=A.mult)
                            tr1 = mac.tile([P, CH * NP, 2, D], f16, tag="tr1")
                            nc.vector.tensor_tensor(
                                out=tr1[:, 0:nt_c * NP, :, :],
                                in0=_v(prod[:], 0,
                                       [[128, nt_c * NP], [64, 2], [1, D]]),
                                in1=_v(prod[:], 32,
                                       [[128, nt_c * NP], [64, 2], [1, D]]),
                                op=A.add)
                            tr2 = mac.tile([P, CH * NP, D], f16, tag="tr2")
                            nc.vector.tensor_tensor(
                                out=tr2[:, 0:nt_c * NP, :],
                                in0=_v(tr1[:], 0, [[64, nt_c * NP], [1, D]]),
                                in1=_v(tr1[:], 32, [[64, nt_c * NP], [1, D]]),
                                op=A.add)
                            pa = mac.tile([P, CH, 2, D], f16, tag="pa")
                            nc.vector.tensor_tensor(
                                out=pa[:, 0:nt_c, :, :],
                                in0=_v(tr2[:], 0,
                                       [[128, nt_c], [32, 2], [1, D]]),
                                in1=_v(tr2[:], 64,
                                       [[128, nt_c], [32, 2], [1, D]]),
                                op=A.add)
                            accv = _v(acc[:], t0 * M * D + m * D,
                                      [[M * D, nt_c], [1, D]])
                            if l == 0:
                                nc.vector.tensor_tensor(
                                    out=accv,
                                    in0=_v(pa[:], 0, [[64, nt_c], [1, D]]),
                                    in1=_v(pa[:], 32, [[64, nt_c], [1, D]]),
                                    op=A.add)
                            else:
                                pb2 = mac.tile([P, CH, D], f16, tag="pb2")
                                nc.vector.tensor_tensor(
                                    out=pb2[:, 0:nt_c, :],
                                    in0=_v(pa[:], 0, [[64, nt_c], [1, D]]),
                                    in1=_v(pa[:], 32, [[64, nt_c], [1, D]]),
                                    op=A.add)
                                nc.vector.tensor_tensor(
                                    out=accv, in0=accv,
                                    in1=pb2[:, 0:nt_c, :], op=A.add)

                # ---------- phase D ----------
                for t in range(NT):
                    at1 = pb.tile([P, P], f32, tag="at1")
                    at2 = pb.tile([P, P], f32, tag="at2")
                    for hh, att in ((0, at1), (1, at2)):
                        tp = pstA.tile([P, P], f32, tag="psT")
                        nc.tensor.transpose(
                            tp[:],
                            _v(acc[:], t * M * D + hh * P, [[1, P]]),
                            id32[:])
                        nc.scalar.activation(att[:], tp[:], AF.Copy)
                    psO = pso.tile([P, C], f32, tag="psO")
                    nc.tensor.matmul(psO[:], at1[:], wo_sb[:, 0, :],
                                     start=True, stop=False)
                    nc.tensor.matmul(psO[:], at2[:], wo_sb[:, 1, :],
                                     start=False, stop=False)
                    nc.tensor.matmul(psO[:], ones1[:], bo_sb[:],
                                     start=False, stop=True)
                    ot = pb.tile([P, C], f32, tag="ot")
                    nc.scalar.activation(ot[:], psO[:], AF.Copy)
                    nc.sync.dma_start(out[t * P:(t + 1) * P, :], ot[:])

    nc.compile()
    return nc


_NC_LOCK = threading.Lock()
_NC = None


def _get_nc():
    global _NC
    with _NC_LOCK:
        if _NC is None:
            _NC = build_nc()
    return _NC


def make_core_inputs(query, reference_points, input_flatten, W_off, b_off,
                     W_attn, b_attn, W_val, b_val, W_out, b_out):
    """Host-side prep: returns list of 8 per-core in_maps."""
    W_off = np.asarray(W_off, np.float32)
    b_off = np.asarray(b_off, np.float32)
    W_attn = np.asarray(W_attn, np.float32)
    b_attn = np.asarray(b_attn, np.float32)
    wc = np.zeros((264, 384), np.float32)
    wcb = np.zeros((1, 384), np.float32)
    ml, pl = np.arange(M), np.arange(NP)
    for l, (h, w) in enumerate(SPATIAL):
        for xy in range(2):
            j = l * 64 + xy * 32 + ml[:, None] * 4 + pl[None, :]
            jp = ml[:, None] * 32 + l * 8 + pl[None, :] * 2 + xy
            wc[0:256, j.ravel()] = W_off[:, jp.ravel()]
            wc[256 + l * 2 + xy, j.ravel()] = float(w if xy == 0 else h)
            wcb[0, j.ravel()] = b_off[jp.ravel()] - 0.5
        j = 256 + l * 32 + ml[:, None] * 4 + pl[None, :]
        jp = ml[:, None] * 16 + l * 4 + pl[None, :]
        wc[0:256, j.ravel()] = W_attn[:, jp.ravel()]
        wcb[0, j.ravel()] = b_attn[jp.ravel()]
    shared = {
        "wc": np.ascontiguousarray(wc),
        "wcb": wcb,
        "wv": np.ascontiguousarray(np.asarray(W_val, np.float32)),
        "bv": np.ascontiguousarray(np.asarray(b_val, np.float32)[None]),
        "wo": np.ascontiguousarray(np.asarray(W_out, np.float32)),
        "bo": np.ascontiguousarray(np.asarray(b_out, np.float32)[None]),
    }
    ias = [np.ascontiguousarray(np.asarray(input_flatten[n], np.float32).T)
           for n in range(N)]
    in_maps = []
    for core in range(8):
        n, qsl = core // 4, core % 4
        s0 = qsl * QS
        qam = np.zeros((264, QP), np.float32)
        qam[0:256, 0:QS] = np.asarray(query[n, s0:s0 + QS], np.float32).T
        ref = np.asarray(reference_points[n, s0:s0 + QS], np.float32)
        qam[256:264, 0:QS] = ref.reshape(QS, 8).T
        in_maps.append({"qa": qam, "ia": ias[n], **shared})
    return in_maps


def kernel(**inputs):
    from concourse.bass_utils import run_bass_kernel_spmd
    nc = _get_nc()
    in_maps = make_core_inputs(**inputs)
    res = run_bass_kernel_spmd(nc, in_maps, core_ids=list(range(8)))
    OUT = np.empty((N, LQ, C), np.float32)
    for core in range(8):
        n, qsl = core // 4, core % 4
        OUT[n, qsl * QS:(qsl + 1) * QS] = res.results[core]["out"][:QS]
    return OUT



# revision 2
# speedup vs baseline: 3.5218x; 3.5218x over previous
"""MS-Deformable-Attention Trainium2 kernel.

Sharding: 2 cores, one full batch (Lq=21760 queries) per core. The run is
transfer-bound through the axon tunnel, so the layout minimizes host<->device
bytes: query/input_flatten/weights ship as fp16, reference points as fp32
(bilinear weights are first-order sensitive to coordinate error), output
returns as fp16. No cross-core communication.

Per-core pipeline:
  A) value = input_flatten @ W_val + b_val (PE, fp16), then build v4 tables:
     for each (head m, level l, cell i) a 256B fp16 row holding all 4
     bilinear corner values [k=4, ch=32] (corner shifts via identity-sliced
     PE matmuls). Levels 0/1 are per-head tables, levels 2/3 pack all heads
     (so every dma_gather int16 index stays < 32768).
  B) offsets/attn projection (PE; fp16 query x fp16 weights plus fp32
     ref-points x fp32 scale rows accumulated in one PSUM group), softmax
     (ACT exp + DVE reduce), pixel coords -> clamped corner base, fractional
     corner weights, gather indices. Indices are folded into dma_gather's
     16-partition-wrapped int16 layout with two PE transposes per q-tile,
     then replicated across the 8 Q7 groups.
  C) one 256B dma_gather row per sample (q,m,l,p), then weighted corner
     reduce on DVE (broadcast mult + pair-add tree) into a per-chunk acc.
  D) per chunk: PE transpose of acc; out = accT.T @ W_out + b_out (fp16);
     DMA out as fp16.

Gather order per (m,l,chunk): i = 128*(4*tl + p) + r  (r = q%128), so
gathered row i lands at out[r, 4*tl+p] and the idx tensor holds idx(r,tl,p)
at [r%16, 8*(4*tl+p) + r//16] -- exactly the (i%16, i//16) wrap.
"""
import os
import threading

import numpy as np

try:  # persistent XLA compilation cache: skips recompile on warm calls
    import jax

    jax.config.update("jax_compilation_cache_dir",
                      os.path.expanduser("~/.jax_comp_cache"))
    jax.config.update("jax_persistent_cache_min_compile_time_secs", 0.0)
    jax.config.update("jax_persistent_cache_min_entry_size_bytes", -1)
except Exception:
    pass

import concourse.bass as bass
import concourse.mybir as mybir
import concourse.tile as tile
import concourse.bacc as bacc
from concourse.masks import make_identity
from contextlib import ExitStack

SPATIAL = ((128, 128), (64, 64), (32, 32), (16, 16))
STARTS = (0, 16384, 20480, 21504)
NBLK = (128, 32, 8, 2)            # 128-row blocks per level
M, L, NP, D, C = 8, 4, 4, 32, 256
LQ, N = 21760, 2
NT = 170                          # q-tiles (21760 = 170*128, no padding)
CH = 4                            # q-tiles per phase-C chunk
NCHUNK = (NT + CH - 1) // CH      # 43
GBLK = 170
P = 128
f32, f16, i16 = mybir.dt.float32, mybir.dt.float16, mybir.dt.int16
A = mybir.AluOpType
AF = mybir.ActivationFunctionType


def _b(ap, n):
    """Append a broadcast (step-0) innermost dim of size n to an AP."""
    return bass.AP(ap.tensor, ap.offset, list(ap.ap) + [[0, n]])


def _v(ap, off, dims):
    """SBUF view: keep partition dim, replace free dims ([step, count] in
    elements), add `off` elements."""
    return bass.AP(ap.tensor, ap.offset + off,
                   [list(ap.ap[0])] + [list(d) for d in dims])


def _dv(ap, off, dims):
    """DRAM view: replace ALL dims."""
    return bass.AP(ap.tensor, ap.offset + off, [list(d) for d in dims])


def _pv(ap, p0, pcnt, off, dims):
    """SBUF view with partition sub-range [p0, p0+pcnt) and explicit free
    dims."""
    pstep = ap.ap[0][0]
    return bass.AP(ap.tensor, ap.offset + p0 * pstep + off,
                   [[pstep, pcnt]] + [list(d) for d in dims])


def _glvl(g):
    for l in range(L):
        if g < STARTS[l] // P + NBLK[l]:
            return l, g - STARTS[l] // P
    raise ValueError(g)


def build_nc():
    nc = bacc.Bacc("TRN2")
    with tile.TileContext(nc) as tc:
        with tc.tile_pool(name="dram", bufs=1, space="DRAM") as dram:
            qaf = dram.tile([C, LQ], f16, kind="ExternalInput", uniquify=False, name="qaf")
            ref = dram.tile([8, LQ], f32, kind="ExternalInput", uniquify=False, name="ref")
            iaf = dram.tile([C, LQ], f16, kind="ExternalInput", uniquify=False, name="iaf")
            wc = dram.tile([C, 384], f16, kind="ExternalInput", uniquify=False, name="wc")
            wcr = dram.tile([8, 384], f32, kind="ExternalInput", uniquify=False, name="wcr")
            wcb = dram.tile([1, 384], f32, kind="ExternalInput", uniquify=False, name="wcb")
            wv = dram.tile([C, C], f16, kind="ExternalInput", uniquify=False, name="wv")
            bv = dram.tile([1, C], f32, kind="ExternalInput", uniquify=False, name="bv")
            wo = dram.tile([C, C], f16, kind="ExternalInput", uniquify=False, name="wo")
            bo = dram.tile([1, C], f32, kind="ExternalInput", uniquify=False, name="bo")
            out = dram.tile([LQ, C], f16, kind="ExternalOutput", uniquify=False, name="out")
            # gather tables: l0/l1 per-head, l2/l3 all-head (int16 idx limit)
            v4t01 = [[dram.tile([SPATIAL[l][0] * SPATIAL[l][1], 128], f16,
                                uniquify=False, name=f"v4_{l}_{m}")
                      for m in range(M)] for l in (0, 1)]
            v4t23 = [dram.tile([M * SPATIAL[l][0] * SPATIAL[l][1], 128], f16,
                               uniquify=False, name=f"v4_{l}")
                     for l in (2, 3)]

            def v4_tensor(l, m):
                return v4t01[l][m] if l < 2 else v4t23[l - 2]

            with ExitStack() as ctx:
                const = ctx.enter_context(tc.tile_pool(name="const", bufs=1))
                io = ctx.enter_context(tc.tile_pool(name="io", bufs=3))
                vwin = ctx.enter_context(tc.tile_pool(name="vwin", bufs=4))
                v4p = ctx.enter_context(tc.tile_pool(name="v4p", bufs=2))
                pb = ctx.enter_context(tc.tile_pool(name="pb", bufs=2))
                w4p = ctx.enter_context(tc.tile_pool(name="w4p", bufs=2))
                gb = ctx.enter_context(tc.tile_pool(name="gb", bufs=3))
                mac = ctx.enter_context(tc.tile_pool(name="mac", bufs=2))
                accp = ctx.enter_context(tc.tile_pool(name="accp", bufs=2))
                psv = ctx.enter_context(tc.tile_pool(name="psv", bufs=1, space="PSUM"))
                psv4 = ctx.enter_context(tc.tile_pool(name="psv4", bufs=1, space="PSUM"))
                psb = ctx.enter_context(tc.tile_pool(name="psb", bufs=2, space="PSUM"))
                pstA = ctx.enter_context(tc.tile_pool(name="pstA", bufs=1, space="PSUM"))
                pstB = ctx.enter_context(tc.tile_pool(name="pstB", bufs=2, space="PSUM"))
                pso = ctx.enter_context(tc.tile_pool(name="pso", bufs=1, space="PSUM"))

                # ---------- constants ----------
                id16 = const.tile([P, P], f16)
                make_identity(nc, id16)
                id32 = const.tile([P, P], f32)
                make_identity(nc, id32)
                ones1 = const.tile([1, P], f32)
                nc.any.memset(ones1[:], 1.0)
                cpos1 = const.tile([P, 1], f32)
                nc.any.memset(cpos1[:], 1.0)
                cneg1 = const.tile([P, 1], f32)
                nc.any.memset(cneg1[:], -1.0)

                # shifted identities for the corner-shift matmuls:
                # SH[poff][k,p]=1 iff k=p+poff ; SH2[poff][k,p]=1 iff
                # k=p-(128-poff) (the next-block wrap part)
                shm, shm2 = {}, {}
                for poff in (1, 16, 17, 32, 33, 64, 65):
                    s1 = const.tile([P, P], f16, name=f"sh_{poff}")
                    nc.gpsimd.memset(s1[:], 0.0)
                    nc.gpsimd.affine_select(
                        out=s1[:], in_=s1[:],
                        compare_op=A.not_equal, fill=1.0, base=-poff,
                        pattern=[[-1, P]], channel_multiplier=1)
                    shm[poff] = s1
                    s2 = const.tile([P, P], f16, name=f"sh2_{poff}")
                    nc.gpsimd.memset(s2[:], 0.0)
                    nc.gpsimd.affine_select(
                        out=s2[:], in_=s2[:],
                        compare_op=A.not_equal, fill=1.0, base=P - poff,
                        pattern=[[-1, P]], channel_multiplier=1)
                    shm2[poff] = s2

                lim = const.tile([P, 256], f32)
                for l, (h, w) in enumerate(SPATIAL):
                    nc.any.memset(lim[:, l * 64: l * 64 + 32], float(w - 2))
                    nc.any.memset(lim[:, l * 64 + 32: l * 64 + 64], float(h - 2))
                # head offset into combined l2/l3 tables per idx-col (l,m,p)
                moff = const.tile([P, 128], f32)
                nc.any.memset(moff[:, 0:64], 0.0)
                for l in (2, 3):
                    h, w = SPATIAL[l]
                    for m in range(M):
                        nc.any.memset(
                            moff[:, l * 32 + m * 4: l * 32 + m * 4 + 4],
                            float(m * h * w))

                wv_sb = const.tile([P, 2, C], f16)
                nc.sync.dma_start(wv_sb[:, 0, :], wv[0:128, :])
                nc.sync.dma_start(wv_sb[:, 1, :], wv[128:256, :])
                bv_sb = const.tile([1, C], f32)
                nc.sync.dma_start(bv_sb[:], bv[:])
                wc_sb = const.tile([P, 2, 384], f16)
                nc.sync.dma_start(wc_sb[:, 0, :], wc[0:128, :])
                nc.sync.dma_start(wc_sb[:, 1, :], wc[128:256, :])
                wcr_sb = const.tile([8, 384], f32)
                nc.sync.dma_start(wcr_sb[:], wcr[:])
                wcb_sb = const.tile([1, 384], f32)
                nc.sync.dma_start(wcb_sb[:], wcb[:])
                wo_sb = const.tile([P, 2, C], f16)
                nc.sync.dma_start(wo_sb[:, 0, :], wo[0:128, :])
                nc.sync.dma_start(wo_sb[:, 1, :], wo[128:256, :])
                bo_sb = const.tile([1, C], f32)
                nc.sync.dma_start(bo_sb[:], bo[:])

                # ---------- phase A: value proj + v4 build ----------
                vts = []

                def compute_value(b):
                    it = io.tile([P, 2, P], f16, tag="ia_t")
                    nc.sync.dma_start(it[:, 0, :], iaf[0:128, b * P:(b + 1) * P])
                    nc.sync.dma_start(it[:, 1, :], iaf[128:256, b * P:(b + 1) * P])
                    ps = psv.tile([P, C], f32, tag="psV")
                    nc.tensor.matmul(ps[:], it[:, 0, :], wv_sb[:, 0, :],
                                     start=True, stop=False)
                    nc.tensor.matmul(ps[:], it[:, 1, :], wv_sb[:, 1, :],
                                     start=False, stop=False)
                    nc.tensor.matmul(ps[:], ones1[:], bv_sb[:],
                                     start=False, stop=True)
                    vt = vwin.tile([P, C], f16, tag="vt")
                    nc.scalar.activation(vt[:], ps[:], AF.Copy)
                    vts.append(vt)

                v4g_holder = [None]

                def build_v4(g):
                    """Fill v4 group tile for global block g; emit group DMA
                    when the group completes."""
                    l, j = _glvl(g)
                    h, w = SPATIAL[l]
                    grp = 4 if l < 2 else 1
                    bl = j % grp
                    if bl == 0:
                        v4g_holder[0] = v4p.tile([P, grp, M, 4, D], f16,
                                                 tag="v4g", name="v4g")
                    v4g = v4g_holder[0]
                    for half in range(2):
                        ps4 = psv4.tile([P, 4, 4, D], f32, tag="ps4")
                        rhs_cols = slice(half * 4 * D, half * 4 * D + 4 * D)
                        for k, (ky, kx) in enumerate(
                                [(0, 0), (0, 1), (1, 0), (1, 1)]):
                            dlt = ky * w + kx
                            joff, poff = divmod(dlt, P)
                            src1 = vts[min(g + joff, GBLK - 1)]
                            if poff == 0:
                                nc.tensor.matmul(
                                    ps4[:, k, :, :], id16[:],
                                    src1[:, rhs_cols],
                                    start=True, stop=True)
                            else:
                                src2 = vts[min(g + joff + 1, GBLK - 1)]
                                nc.tensor.matmul(
                                    ps4[:, k, :, :], shm[poff][:],
                                    src1[:, rhs_cols],
                                    start=True, stop=False)
                                nc.tensor.matmul(
                                    ps4[:, k, :, :], shm2[poff][:],
                                    src2[:, rhs_cols],
                                    start=False, stop=True)
                        # psum (k, m4, ch) -> group tile (m4, k, ch)
                        nc.scalar.activation(
                            _v(v4g[:], bl * M * 4 * D + half * 4 * 4 * D,
                               [[4 * D, 4], [D, 4], [1, D]]),
                            _v(ps4[:], 0, [[D, 4], [4 * D, 4], [1, D]]),
                            AF.Copy)
                    if bl == grp - 1:
                        j0 = j - bl
                        if l < 2:
                            for m in range(M):
                                nc.sync.dma_start(
                                    _dv(v4_tensor(l, m)[:], j0 * P * P,
                                        [[P, P], [P * P, grp], [1, 4 * D]]),
                                    _v(v4g[:], m * 4 * D,
                                       [[M * 4 * D, grp], [1, 4 * D]]))
                        else:
                            nc.sync.dma_start(
                                _dv(v4_tensor(l, 0)[:], j0 * P * P,
                                    [[P, P], [h * w * P, M], [1, 4 * D]]),
                                _v(v4g[:], 0, [[4 * D, M], [1, 4 * D]]))

                for g in range(GBLK):
                    compute_value(g)
                    if g >= 2:
                        build_v4(g - 2)
                build_v4(GBLK - 2)
                build_v4(GBLK - 1)

                # ---------- phases B/C/D interleaved per chunk ----------
                for c in range(NCHUNK):
                    t0, t1n = c * CH, min((c + 1) * CH, NT)
                    nt_c = t1n - t0
                    # idxw: 16-wrapped idx tensor, replicated to 128 parts.
                    # free layout (l, m, tl, p, u): per-(m,l) slice contiguous
                    idxm = w4p.tile([16, L * M * CH * NP * 8], i16, tag="idxm")
                    idxw = w4p.tile([P, L, M, CH, NP, 8], i16, tag="idxw")
                    if nt_c < CH:
                        nc.vector.memset(idxm[:], 0)
                    w4c = w4p.tile([P, CH, 128, 4], f16, tag="w4c")

                    for t in range(t0, t1n):
                        tl = t - t0
                        cs = slice(t * P, (t + 1) * P)
                        qt = io.tile([P, 2, P], f16, tag="qa_t")
                        nc.sync.dma_start(qt[:, 0, :], qaf[0:128, cs])
                        nc.sync.dma_start(qt[:, 1, :], qaf[128:256, cs])
                        rt = io.tile([8, P], f32, tag="ref_t")
                        nc.sync.dma_start(rt[:], ref[:, cs])

                        psB = psb.tile([P, 384], f32, tag="psB")
                        nc.tensor.matmul(psB[:], qt[:, 0, :], wc_sb[:, 0, :],
                                         start=True, stop=False)
                        nc.tensor.matmul(psB[:], qt[:, 1, :], wc_sb[:, 1, :],
                                         start=False, stop=False)
                        nc.tensor.matmul(psB[:], rt[:], wcr_sb[:],
                                         start=False, stop=False)
                        nc.tensor.matmul(psB[:], ones1[:], wcb_sb[:],
                                         start=False, stop=True)

                        # softmax over (l,p) per m; attn cols 256+(l,m,p)
                        ex = pb.tile([P, 128], f32, tag="ex")
                        nc.scalar.activation(ex[:], psB[:, 256:384], AF.Exp)
                        r1 = pb.tile([P, 32], f32, tag="r1")  # (l,m)
                        nc.vector.tensor_reduce(
                            out=r1[:],
                            in_=_v(ex[:], 0, [[32, 4], [4, 8], [1, 4]]),
                            op=A.add, axis=mybir.AxisListType.X)
                        r2 = pb.tile([P, 8], f32, tag="r2")   # (m)
                        nc.vector.tensor_reduce(
                            out=r2[:], in_=_v(r1[:], 0, [[1, 8], [8, 4]]),
                            op=A.add, axis=mybir.AxisListType.X)
                        rc = pb.tile([P, 8], f32, tag="rc")
                        nc.vector.reciprocal(rc[:], r2[:])
                        at = pb.tile([P, 128], f32, tag="at")  # (l,m,p)
                        nc.vector.tensor_tensor(
                            out=at[:], in0=ex[:],
                            in1=_v(rc[:], 0, [[0, 4], [1, 8], [0, 4]]),
                            op=A.mult)

                        # coords: cols (l,xy,m,p)
                        xcl = pb.tile([P, 256], f32, tag="xcl")
                        nc.vector.tensor_scalar(
                            out=xcl[:], in0=psB[:, 0:256], scalar1=0.0,
                            scalar2=None, op0=A.max)
                        xc = pb.tile([P, 256], f32, tag="xc")
                        nc.vector.tensor_tensor(out=xc[:], in0=xcl[:],
                                                in1=lim[:], op=A.min)
                        # floor(xc) via round-to-nearest (+-2^23) then
                        # fix-up: bs0 > xc  =>  bs0 - 1
                        bs0 = pb.tile([P, 256], f32, tag="bs0")
                        nc.vector.tensor_scalar(
                            out=bs0[:], in0=xc[:], scalar1=8388608.0,
                            scalar2=-8388608.0, op0=A.add, op1=A.add)
                        cmpg = pb.tile([P, 256], f32, tag="cmpg")
                        nc.vector.tensor_tensor(out=cmpg[:], in0=bs0[:],
                                                in1=xc[:], op=A.is_gt)
                        bs = pb.tile([P, 256], f32, tag="bs")
                        nc.vector.tensor_tensor(out=bs[:], in0=bs0[:],
                                                in1=cmpg[:], op=A.subtract)
                        f = pb.tile([P, 256], f32, tag="f")
                        nc.vector.tensor_tensor(out=f[:], in0=psB[:, 0:256],
                                                in1=bs[:], op=A.subtract)
                        ab0 = pb.tile([P, 256], f32, tag="ab0")
                        nc.scalar.activation(ab0[:], f[:], AF.Abs)
                        w0 = pb.tile([P, 256], f32, tag="w0")
                        nc.scalar.activation(w0[:], ab0[:], AF.Relu,
                                             scale=-1.0, bias=cpos1[:])
                        ab1 = pb.tile([P, 256], f32, tag="ab1")
                        nc.scalar.activation(ab1[:], f[:], AF.Abs,
                                             bias=cneg1[:])
                        w1 = pb.tile([P, 256], f32, tag="w1")
                        nc.scalar.activation(w1[:], ab1[:], AF.Relu,
                                             scale=-1.0, bias=cpos1[:])

                        byw = pb.tile([P, 128], f32, tag="byw")
                        for l, (h, w) in enumerate(SPATIAL):
                            nc.vector.tensor_scalar(
                                out=byw[:, l * 32:(l + 1) * 32],
                                in0=bs[:, l * 64 + 32: l * 64 + 64],
                                scalar1=float(w), scalar2=None, op0=A.mult)
                        idxf = pb.tile([P, 128], f32, tag="idxf")
                        nc.vector.tensor_tensor(
                            out=idxf[:],
                            in0=_v(bs[:], 0, [[64, 4], [1, 32]]),
                            in1=byw[:], op=A.add)
                        idxf2 = pb.tile([P, 128], f32, tag="idxf2")
                        nc.vector.tensor_tensor(out=idxf2[:], in0=idxf[:],
                                                in1=moff[:], op=A.add)

                        # fold idx into 16-wrap: transpose, then 8 16-col
                        # slice transposes put r%16 on partitions
                        psT = pstA.tile([P, P], f32, tag="psT")
                        nc.tensor.transpose(psT[:], idxf2[:], id32[:])
                        idxT = pb.tile([P, P], f32, tag="idxT")
                        nc.scalar.activation(idxT[:], psT[:], AF.Copy)
                        for u in range(8):
                            psU = pstB.tile([16, P], f32, tag="psU")
                            nc.tensor.transpose(
                                psU[:], idxT[:, 16 * u:16 * (u + 1)], id32[:])
                            # psU[r2,(l,m,p)] -> idxm[r2, (l, m, tl, p, u)]
                            nc.vector.tensor_copy(
                                out=_pv(idxm[:], 0, 16, tl * NP * 8 + u,
                                        [[M * CH * NP * 8, 4],
                                         [CH * NP * 8, 8], [8, 4]]),
                                in_=_v(psU[:], 0, [[32, 4], [4, 8], [1, 4]]))

                        # w4[(l,m,p), k=(ky,kx)] = wy_ky*wx_kx*attn (fp16)
                        for ky in range(2):
                            wyt = (w0, w1)[ky]
                            for kx in range(2):
                                wxt = (w0, w1)[kx]
                                tmp = pb.tile([P, 128], f32, tag="tmp")
                                nc.vector.tensor_tensor(
                                    out=tmp[:],
                                    in0=_v(wyt[:], 32, [[64, 4], [1, 32]]),
                                    in1=_v(wxt[:], 0, [[64, 4], [1, 32]]),
                                    op=A.mult)
                                nc.vector.tensor_tensor(
                                    out=_v(w4c[:], tl * 512 + ky * 2 + kx,
                                           [[4, 128]]),
                                    in0=tmp[:], in1=at[:], op=A.mult)

                    # replicate idx wrap to all 8 Q7 partition groups
                    for r in range(8):
                        nc.sync.dma_start(
                            _pv(idxw[:], 16 * r, 16, 0, [[1, 4096]]),
                            _pv(idxm[:], 0, 16, 0, [[1, 4096]]))

                    # ---- phase C: gathers + weighted reduce ----
                    acc = accp.tile([P, CH, M, D], f32, tag="acc")
                    for l in range(L):
                        for m in range(M):
                            gbuf = gb.tile([P, CH * NP, 128], f16, tag="gbuf")
                            # Q7 idx scratch caps num_idxs at 1024/call
                            for hfi in range(0, nt_c, 2):
                                nth = min(2, nt_c - hfi)
                                nih = nth * 512
                                nc.gpsimd.dma_gather(
                                    gbuf[:, hfi * NP:(hfi + nth) * NP, :],
                                    v4_tensor(l, m)[:],
                                    _v(idxw[:],
                                       (l * M + m) * CH * NP * 8 + hfi * 32,
                                       [[1, nth * 32]]),
                                    nih, nih, 128)
                            prod = mac.tile([P, CH * NP, 4, D], f16, tag="prod")
                            nc.vector.tensor_tensor(
                                out=_v(prod[:], 0,
                                       [[512, nt_c], [128, 4], [32, 4], [1, D]]),
                                in0=_v(gbuf[:], 0,
                                       [[512, nt_c], [128, 4], [32, 4], [1, D]]),
                                in1=_b(_v(w4c[:], (l * 32 + m * 4) * 4,
                                          [[512, nt_c], [4, 4], [1, 4]]), D),
                                op=A.mult)
                            tr1 = mac.tile([P, CH * NP, 2, D], f16, tag="tr1")
                            nc.vector.tensor_tensor(
                                out=tr1[:, 0:nt_c * NP, :, :],
                                in0=_v(prod[:], 0,
                                       [[128, nt_c * NP], [64, 2], [1, D]]),
                                in1=_v(prod[:], 32,
                                       [[128, nt_c * NP], [64, 2], [1, D]]),
                                op=A.add)
                            tr2 = mac.tile([P, CH * NP, D], f16, tag="tr2")
                            nc.vector.tensor_tensor(
                                out=tr2[:, 0:nt_c * NP, :],
                                in0=_v(tr1[:], 0, [[64, nt_c * NP], [1, D]]),
                                in1=_v(tr1[:], 32, [[64, nt_c * NP], [1, D]]),
                                op=A.add)
                            pa = mac.tile([P, CH, 2, D], f16, tag="pa")
                            nc.vector.tensor_tensor(
                                out=pa[:, 0:nt_c, :, :],
                                in0=_v(tr2[:], 0,
                                       [[128, nt_c], [32, 2], [1, D]]),
                                in1=_v(tr2[:], 64,
                                       [[128, nt_c], [32, 2], [1, D]]),
                                op=A.add)
                            accv = _v(acc[:], m * D,
                                      [[M * D, nt_c], [1, D]])
                            if l == 0:
                                nc.vector.tensor_tensor(
                                    out=accv,
                                    in0=_v(pa[:], 0, [[64, nt_c], [1, D]]),
                                    in1=_v(pa[:], 32, [[64, nt_c], [1, D]]),
                                    op=A.add)
                            else:
                                pb2 = mac.tile([P, CH, D], f16, tag="pb2")
                                nc.vector.tensor_tensor(
                                    out=pb2[:, 0:nt_c, :],
                                    in0=_v(pa[:], 0, [[64, nt_c], [1, D]]),
                                    in1=_v(pa[:], 32, [[64, nt_c], [1, D]]),
                                    op=A.add)
                                nc.vector.tensor_tensor(
                                    out=accv, in0=accv,
                                    in1=pb2[:, 0:nt_c, :], op=A.add)

                    # ---- phase D: output projection for this chunk ----
                    for t in range(t0, t1n):
                        tl = t - t0
                        at1 = pb.tile([P, P], f16, tag="at1")
                        at2 = pb.tile([P, P], f16, tag="at2")
                        for hh, att in ((0, at1), (1, at2)):
                            tp = pstA.tile([P, P], f32, tag="psT")
                            nc.tensor.transpose(
                                tp[:],
                                _v(acc[:], tl * M * D + hh * P, [[1, P]]),
                                id32[:])
                            nc.scalar.activation(att[:], tp[:], AF.Copy)
                        psO = pso.tile([P, C], f32, tag="psO")
                        nc.tensor.matmul(psO[:], at1[:], wo_sb[:, 0, :],
                                         start=True, stop=False)
                        nc.tensor.matmul(psO[:], at2[:], wo_sb[:, 1, :],
                                         start=False, stop=False)
                        nc.tensor.matmul(psO[:], ones1[:], bo_sb[:],
                                         start=False, stop=True)
                        ot = pb.tile([P, C], f16, tag="ot")
                        nc.scalar.activation(ot[:], psO[:], AF.Copy)
                        nc.sync.dma_start(out[t * P:(t + 1) * P, :], ot[:])

    nc.compile()
    return nc


_NC_LOCK = threading.Lock()
_NC = None


def _get_nc():
    global _NC
    with _NC_LOCK:
        if _NC is None:
            _NC = build_nc()
    return _NC


def make_core_inputs(query, reference_points, input_flatten, W_off, b_off,
                     W_attn, b_attn, W_val, b_val, W_out, b_out):
    """Host-side prep: returns list of 2 per-core in_maps (core = batch)."""
    W_off = np.asarray(W_off, np.float32)
    b_off = np.asarray(b_off, np.float32)
    W_attn = np.asarray(W_attn, np.float32)
    b_attn = np.asarray(b_attn, np.float32)
    wc = np.zeros((C, 384), np.float32)
    wcr = np.zeros((8, 384), np.float32)
    wcb = np.zeros((1, 384), np.float32)
    ml, pl = np.arange(M), np.arange(NP)
    for l, (h, w) in enumerate(SPATIAL):
        for xy in range(2):
            j = l * 64 + xy * 32 + ml[:, None] * 4 + pl[None, :]
            jp = ml[:, None] * 32 + l * 8 + pl[None, :] * 2 + xy
            wc[:, j.ravel()] = W_off[:, jp.ravel()]
            wcr[l * 2 + xy, j.ravel()] = float(w if xy == 0 else h)
            wcb[0, j.ravel()] = b_off[jp.ravel()] - 0.5
        j = 256 + l * 32 + ml[:, None] * 4 + pl[None, :]
        jp = ml[:, None] * 16 + l * 4 + pl[None, :]
        wc[:, j.ravel()] = W_attn[:, jp.ravel()]
        wcb[0, j.ravel()] = b_attn[jp.ravel()]
    shared = {
        "wc": np.ascontiguousarray(wc.astype(np.float16)),
        "wcr": np.ascontiguousarray(wcr),
        "wcb": wcb,
        "wv": np.ascontiguousarray(np.asarray(W_val, np.float16)),
        "bv": np.ascontiguousarray(np.asarray(b_val, np.float32)[None]),
        "wo": np.ascontiguousarray(np.asarray(W_out, np.float16)),
        "bo": np.ascontiguousarray(np.asarray(b_out, np.float32)[None]),
    }
    in_maps = []
    for n in range(N):
        qaf = np.ascontiguousarray(
            np.asarray(query[n], np.float16).T)
        rf = np.ascontiguousarray(
            np.asarray(reference_points[n], np.float32).reshape(LQ, 8).T)
        ia = np.ascontiguousarray(
            np.asarray(input_flatten[n], np.float16).T)
        in_maps.append({"qaf": qaf, "ref": rf, "iaf": ia, **shared})
    return in_maps


def kernel(**inputs):
    from concourse.bass_utils import run_bass_kernel_spmd
    nc = _get_nc()
    in_maps = make_core_inputs(**inputs)
    res = run_bass_kernel_spmd(nc, in_maps, core_ids=[0, 1])
    OUT = np.empty((N, LQ, C), np.float32)
    for n in range(N):
        OUT[n] = res.results[n]["out"].astype(np.float32)
    return OUT
